# revision 9
# baseline (speedup 1.0000x reference)
"""DendriticFractalBlock on 8 trn2 NeuronCores (Bass/Tile, SPMD).

Sharding: token-parallel for norm/qkv/o_proj/FFN with each core owning
256 tokens of EACH batch (so AllToAll chunks are all useful);
head-parallel attention (4 heads x 1 batch per core) bridged by two
8-core AllToAll collectives. Local matmuls run fp32r (full PE rate at
free-dim 512); tensors crossing the collectives (q/k/v, attention out)
are bf16.
"""

import numpy as np

import concourse.bass as bass
import concourse.mybir as mybir
import concourse.tile as tile
import concourse.bacc as bacc
from concourse.bass_utils import run_bass_kernel_spmd

F32 = mybir.dt.float32
F32R = mybir.dt.float32r
BF16 = mybir.dt.bfloat16
AF = mybir.ActivationFunctionType
ALU = mybir.AluOpType

N_CORES = 8
RG = [list(range(8))]
B, T, D, FF = 2, 2048, 1024, 2048
ND = 4
TOK = 512          # tokens per core (256 per batch)
HB = 256           # per-batch tokens per core
P = 128
QKV = 3 * D
EPS = float(np.finfo(np.float32).eps)
NEG = -1.0e30

TRACE = False
LAST_EXEC_NS = None
LAST_RESULTS = None

_CACHE = {}


def _build():
    nc = bacc.Bacc("TRN2", target_bir_lowering=False, debug=False,
                   num_devices=N_CORES)

    x_d = nc.dram_tensor("x_shard", [TOK, D], F32, kind="ExternalInput")
    qkvw_d = nc.dram_tensor("qkv_w_r", [8, P, QKV], F32R, kind="ExternalInput")
    gatew_d = nc.dram_tensor("gate_w_r", [8, P, FF], F32R, kind="ExternalInput")
    upw_d = nc.dram_tensor("up_w_r", [8, P, FF], F32R, kind="ExternalInput")
    downw_d = nc.dram_tensor("down_w_r", [16, P, D], F32R, kind="ExternalInput")
    ow_d = nc.dram_tensor("o_w_b", [8, P, D], BF16, kind="ExternalInput")
    nthr_qkv_d = nc.dram_tensor("nthr_qkv", [24, P, ND], F32, kind="ExternalInput")
    gain_qkv_d = nc.dram_tensor("gain_qkv", [24, P, ND], F32, kind="ExternalInput")
    nthr_gate_d = nc.dram_tensor("nthr_gate", [16, P, ND], F32, kind="ExternalInput")
    gain_gate_d = nc.dram_tensor("gain_gate", [16, P, ND], F32, kind="ExternalInput")
    nthr_up_d = nc.dram_tensor("nthr_up", [16, P, ND], F32, kind="ExternalInput")
    gain_up_d = nc.dram_tensor("gain_up", [16, P, ND], F32, kind="ExternalInput")
    n1w_d = nc.dram_tensor("norm1_wT", [8, P, 1], F32, kind="ExternalInput")
    n2w_d = nc.dram_tensor("norm2_wT", [8, P, 1], F32, kind="ExternalInput")
    mask_d = nc.dram_tensor("maskc", [4, P, TOK], F32, kind="ExternalInput")
    ident_d = nc.dram_tensor("ident", [P, P], F32, kind="ExternalInput")
    identb_d = nc.dram_tensor("ident_bf", [P, P], BF16, kind="ExternalInput")
    onesb_d = nc.dram_tensor("ones_bf", [P, 1], BF16, kind="ExternalInput")
    out_d = nc.dram_tensor("out_shard", [TOK, D], F32, kind="ExternalOutput")

    with tile.TileContext(nc) as tc:
        with (
            tc.tile_pool(name="persist", bufs=1) as pp,
            tc.tile_pool(name="hT", bufs=1) as hp,
            tc.tile_pool(name="wsl", bufs=6) as wslp,
            tc.tile_pool(name="wbig", bufs=3) as wbp,
            tc.tile_pool(name="scratch", bufs=2) as scp,
            tc.tile_pool(name="work", bufs=3) as wk,
            tc.tile_pool(name="small", bufs=4) as sm,
            tc.tile_pool(name="attn", bufs=17) as ap,
            tc.tile_pool(name="qk", bufs=2) as qp,
            tc.tile_pool(name="vn", bufs=20) as vp,
            tc.tile_pool(name="psum", bufs=8, space="PSUM") as ps,
            tc.tile_pool(name="dram", bufs=1, space="DRAM") as dp,
        ):
            # ---- constants ----
            ident = pp.tile([P, P], F32, tag="ident")
            nc.sync.dma_start(ident[:], ident_d[:])
            identb = pp.tile([P, P], BF16, tag="identb")
            nc.sync.dma_start(identb[:], identb_d[:])
            onesb = pp.tile([P, 1], BF16, tag="onesb")
            nc.sync.dma_start(onesb[:], onesb_d[:])
            epst = pp.tile([P, 1], F32, tag="epst")
            nc.vector.memset(epst[:], EPS)
            masks = []
            for i in range(4):
                m = pp.tile([P, TOK], F32, tag=f"mask{i}", name=f"mask{i}")
                nc.sync.dma_start(m[:], mask_d[i])
                masks.append(m)

            a2a1_in = dp.tile([8, 768, HB], BF16)
            a2a1_out = dp.tile([8, 768, HB], BF16)
            a2a2_in = dp.tile([8, 256, HB], BF16)
            a2a2_out = dp.tile([8, 256, HB], BF16)

            # ---- rmsnorm + transpose to feature-major ----
            def norm_transpose(src_tiles, w_dram, tag):
                nw = pp.tile([P, 8], F32, tag=f"nw_{tag}", name=f"nw_{tag}")
                nc.sync.dma_start(nw[:], w_dram[:].rearrange("a p b -> p (a b)"))
                hT = [hp.tile([P, TOK], F32R, tag=f"hT{dc}", name=f"hT_{tag}{dc}")
                      for dc in range(8)]
                for mt in range(4):
                    xt = src_tiles[mt]
                    xn = scp.tile([P, D], F32, tag="xn")
                    ssq = sm.tile([P, 1], F32, tag="ssq")
                    nc.scalar.activation(xn[:], xt[:], AF.Square,
                                         accum_out=ssq[:])
                    rms = sm.tile([P, 1], F32, tag="rms")
                    nc.scalar.activation(rms[:], ssq[:], AF.Sqrt,
                                         scale=1.0 / D, bias=epst[:, 0:1])
                    rinv = sm.tile([P, 1], F32, tag="rinv")
                    nc.vector.reciprocal(rinv[:], rms[:])
                    nc.vector.tensor_scalar_mul(xn[:], xt[:], rinv[:, 0:1])
                    for dc in range(8):
                        tp = ps.tile([P, TOK], F32, tag="ps")
                        nc.tensor.transpose(tp[:, 0:P],
                                            xn[:, dc * P:(dc + 1) * P], ident[:])
                        nc.scalar.activation(hT[dc][:, mt * P:(mt + 1) * P],
                                             tp[:, 0:P], AF.Identity,
                                             scale=nw[:, dc:dc + 1])
                return hT

            # ---- one dendritic col-tile: max_n gain*silu(w_n.T@h - thr) ----
            def dend_tile(hT, w_dram, nthr, gain, mt, acc_dt):
                cols = slice(mt * P, (mt + 1) * P)
                acc = wk.tile([P, TOK], acc_dt, tag="dacc", name=f"dacc{mt}")
                for n in range(ND):
                    pt = ps.tile([P, TOK], F32, tag="ps")
                    for kk in range(2):
                        w = wslp.tile([P, P], F32R, tag="wsl")
                        nc.sync.dma_start(w[:], w_dram[2 * n + kk, :, cols])
                        nc.tensor.matmul(pt[:], w[:], hT[2 * n + kk][:],
                                         start=(kk == 0), stop=(kk == 1))
                    st = wk.tile([P, TOK], F32, tag="silu")
                    nc.scalar.activation(st[:], pt[:], AF.Silu,
                                         bias=nthr[:, n:n + 1])
                    if n == 0:
                        nc.vector.tensor_scalar_mul(acc[:], st[:],
                                                    gain[:, n:n + 1])
                    else:
                        g2 = wk.tile([P, TOK], F32, tag="gained")
                        nc.vector.tensor_scalar_mul(g2[:], st[:],
                                                    gain[:, n:n + 1])
                        nc.vector.tensor_tensor(acc[:], acc[:], g2[:],
                                                op=ALU.max)
                return acc

            # ---- phase 1: x load + norm1 ----
            x_t = []
            with nc.named_scope("p1_norm1"):
                for mt in range(4):
                    xt = pp.tile([P, D], F32, tag=f"x{mt}", name=f"x{mt}")
                    nc.sync.dma_start(xt[:], x_d[mt * P:(mt + 1) * P, :])
                    x_t.append(xt)
                h1T = norm_transpose(x_t, n1w_d, "n1")

            # ---- phase 2: qkv dendritic -> a2a1_in ----
            scope_p2 = nc.named_scope("p2_qkv")
            scope_p2.__enter__()
            for mt in range(24):
                nthr = sm.tile([P, ND], F32, tag="nthr", name=f"nthrq{mt}")
                nc.sync.dma_start(nthr[:], nthr_qkv_d[mt])
                gain = sm.tile([P, ND], F32, tag="gain", name=f"gainq{mt}")
                nc.sync.dma_start(gain[:], gain_qkv_d[mt])
                acc = dend_tile(h1T, qkvw_d, nthr, gain, mt, BF16)
                part, loc = mt // 8, mt % 8    # part: 0=q 1=k 2=v
                g = loc // 2
                r0 = 256 * part + (loc % 2) * P
                nc.sync.dma_start(a2a1_in[g, r0:r0 + P, :], acc[:, 0:HB])
                nc.sync.dma_start(a2a1_in[4 + g, r0:r0 + P, :], acc[:, HB:TOK])

            scope_p2.__exit__(None, None, None)
            # ---- phase 3: A2A #1 (q/k/v to head owners) ----
            with nc.named_scope("p3_a2a1"):
                nc.gpsimd.collective_compute(
                "AllToAll", ALU.bypass, replica_groups=RG,
                    ins=[a2a1_in[:].opt()], outs=[a2a1_out[:].opt()],
                )

            # ---- phase 4: attention (4 local heads of one batch) ----
            scope_p4 = nc.named_scope("p4_attn")
            scope_p4.__enter__()
            for h in range(4):
                r = 64 * h
                qT = qp.tile([64, T], BF16, tag="qT", name=f"qT{h}")
                kT = qp.tile([64, T], BF16, tag="kT", name=f"kT{h}")
                for j in range(8):
                    c0 = j * HB
                    nc.sync.dma_start(qT[:, c0:c0 + HB],
                                      a2a1_out[j, r:r + 64, :])
                    nc.sync.dma_start(kT[:, c0:c0 + HB],
                                      a2a1_out[j, 256 + r:256 + r + 64, :])
                vn = []
                for tkt in range(16):
                    vts = sm.tile([64, P], BF16, tag="vts", name=f"vts{tkt}")
                    j, inner = tkt // 2, tkt % 2
                    nc.sync.dma_start(
                        vts[:], a2a1_out[j, 512 + r:512 + r + 64,
                                         inner * P:(inner + 1) * P])
                    tp = ps.tile([P, 64], BF16, tag="ps")
                    nc.tensor.transpose(tp[:], vts[:], identb[0:64, 0:64])
                    vt = vp.tile([P, 65], BF16, tag="vn", name=f"vn{tkt}")
                    nc.scalar.copy(vt[:, 0:64], tp[:])
                    nc.scalar.copy(vt[:, 64:65], onesb[:])
                    vn.append(vt)
                for C in range(4):
                    n_tk = 4 * C + 4
                    tq = slice(C * TOK, (C + 1) * TOK)
                    at = []
                    for tkt in range(n_tk):
                        sp = ps.tile([P, TOK], F32, tag="ps")
                        nc.tensor.matmul(sp[:], kT[:, tkt * P:(tkt + 1) * P],
                                         qT[:, tq], start=True, stop=True)
                        if tkt >= 4 * C:
                            nc.vector.tensor_tensor(sp[:], sp[:],
                                                    masks[tkt - 4 * C][:],
                                                    op=ALU.add)
                        a = ap.tile([P, TOK], BF16, tag="attnT",
                                    name=f"at{C}_{tkt}")
                        nc.scalar.activation(a[:], sp[:], AF.Exp, scale=0.125)
                        at.append(a)
                    po = ps.tile([65, TOK], F32, tag="ps", name=f"po{C}")
                    for tkt in range(n_tk):
                        nc.tensor.matmul(po[:], vn[tkt][:], at[tkt][:],
                                         start=(tkt == 0),
                                         stop=(tkt == n_tk - 1))
                    rden = sm.tile([1, TOK], F32, tag="rden")
                    nc.vector.reciprocal(rden[:], po[64:65, :])
                    rb = wk.tile([64, TOK], F32, tag="rb")
                    nc.gpsimd.partition_broadcast(rb[:], rden[:])
                    on = wk.tile([64, TOK], BF16, tag="onorm")
                    nc.vector.tensor_tensor(on[:], po[0:64, :], rb[:],
                                            op=ALU.mult)
                    nc.sync.dma_start(a2a2_in[2 * C, r:r + 64, :],
                                      on[:, 0:HB])
                    nc.sync.dma_start(a2a2_in[2 * C + 1, r:r + 64, :],
                                      on[:, HB:TOK])

            scope_p4.__exit__(None, None, None)
            # ---- phase 5: A2A #2 (attention out to token owners) ----
            with nc.named_scope("p5_a2a2"):
                nc.gpsimd.collective_compute(
                    "AllToAll", ALU.bypass, replica_groups=RG,
                    ins=[a2a2_in[:].opt()], outs=[a2a2_out[:].opt()],
                )

            # ---- phase 6: o_proj + residual -> xres; norm2 -> h2T ----
            scope_p6 = nc.named_scope("p6_oproj")
            scope_p6.__enter__()
            py = [[None, None] for _ in range(4)]
            for mt in range(4):
                for dn in range(2):
                    py[mt][dn] = ps.tile([P, TOK], F32, tag="ps",
                                         name=f"py{mt}{dn}")
            for s in range(8):
                otl = wbp.tile([P, TOK], BF16, tag="otl", name=f"otl{s}")
                j, r0 = s // 2, (s % 2) * P
                nc.sync.dma_start(otl[:, 0:HB], a2a2_out[j, r0:r0 + P, :])
                nc.sync.dma_start(otl[:, HB:TOK], a2a2_out[4 + j, r0:r0 + P, :])
                owt = wbp.tile([P, D], BF16, tag="ow", name=f"ow{s}")
                nc.sync.dma_start(owt[:], ow_d[s])
                for mt in range(4):
                    for dn in range(2):
                        nc.tensor.matmul(py[mt][dn][:],
                                         otl[:, mt * P:(mt + 1) * P],
                                         owt[:, dn * TOK:(dn + 1) * TOK],
                                         start=(s == 0), stop=(s == 7))
            xres = []
            for mt in range(4):
                xr = pp.tile([P, D], F32, tag=f"xr{mt}", name=f"xr{mt}")
                for dn in range(2):
                    cols = slice(dn * TOK, (dn + 1) * TOK)
                    nc.vector.tensor_tensor(xr[:, cols], py[mt][dn][:],
                                            x_t[mt][:, cols], op=ALU.add)
                xres.append(xr)
            h2T = norm_transpose(xres, n2w_d, "n2")

            scope_p6.__exit__(None, None, None)
            # ---- phase 7: gate/up dendritic (interleaved) -> ffT ----
            scope_p7 = nc.named_scope("p7_ffn")
            scope_p7.__enter__()
            ffT = [hp.tile([P, TOK], F32R, tag=f"ffT{mt}", name=f"ffT{mt}")
                   for mt in range(16)]
            for mt in range(16):
                nthr_g = sm.tile([P, ND], F32, tag="nthr", name=f"nthrg{mt}")
                nc.sync.dma_start(nthr_g[:], nthr_gate_d[mt])
                gain_g = sm.tile([P, ND], F32, tag="gain", name=f"gaing{mt}")
                nc.sync.dma_start(gain_g[:], gain_gate_d[mt])
                acc_g = dend_tile(h2T, gatew_d, nthr_g, gain_g, mt, F32)
                nthr_u = sm.tile([P, ND], F32, tag="nthr", name=f"nthru{mt}")
                nc.sync.dma_start(nthr_u[:], nthr_up_d[mt])
                gain_u = sm.tile([P, ND], F32, tag="gain", name=f"gainu{mt}")
                nc.sync.dma_start(gain_u[:], gain_up_d[mt])
                acc_u = dend_tile(h2T, upw_d, nthr_u, gain_u, mt, F32)
                sg = wk.tile([P, TOK], F32, tag="silu2")
                nc.scalar.activation(sg[:], acc_g[:], AF.Silu)
                nc.vector.tensor_tensor(ffT[mt][:], sg[:], acc_u[:],
                                        op=ALU.mult)

            scope_p7.__exit__(None, None, None)
            # ---- phase 8: down proj + residual -> out ----
            scope_p8 = nc.named_scope("p8_down")
            scope_p8.__enter__()
            pdt = [[None, None] for _ in range(4)]
            for mt in range(4):
                for dn in range(2):
                    pdt[mt][dn] = ps.tile([P, TOK], F32, tag="ps",
                                          name=f"pd{mt}{dn}")
            for s in range(16):
                dwt = wbp.tile([P, D], F32R, tag="dw", name=f"dw{s}")
                nc.sync.dma_start(dwt[:], downw_d[s])
                for mt in range(4):
                    for dn in range(2):
                        nc.tensor.matmul(pdt[mt][dn][:],
                                         ffT[s][:, mt * P:(mt + 1) * P],
                                         dwt[:, dn * TOK:(dn + 1) * TOK],
                                         start=(s == 0), stop=(s == 15))
            for mt in range(4):
                for dn in range(2):
                    cols = slice(dn * TOK, (dn + 1) * TOK)
                    of = wk.tile([P, TOK], F32, tag="outf")
                    nc.vector.tensor_tensor(of[:], pdt[mt][dn][:],
                                            xres[mt][:, cols], op=ALU.add)
                    nc.sync.dma_start(out_d[mt * P:(mt + 1) * P, cols], of[:])
            scope_p8.__exit__(None, None, None)

    nc.compile()
    return nc


def _prep_inputs(x, qkv_w, qkv_thr, qkv_gain, o_w, gate_w, gate_thr, gate_gain,
                 up_w, up_thr, up_gain, down_w, norm1_w, norm2_w):
    import ml_dtypes
    f = np.float32
    bf = ml_dtypes.bfloat16
    x = np.asarray(x, f)

    def slabs(w):
        w = np.asarray(w, f)
        n, dd, wid = w.shape
        return np.ascontiguousarray(w.reshape(n * 2, P, wid))

    def tg(v, n_mt, negate=False):
        v = np.asarray(v, f)
        if negate:
            v = -v
        return np.ascontiguousarray(np.transpose(v.reshape(ND, n_mt, P),
                                                 (1, 2, 0)))

    maskc = np.zeros((4, P, TOK), f)
    for p in range(4):
        tk = P * p + np.arange(P)[:, None]
        tq = np.arange(TOK)[None, :]
        maskc[p] = np.where(tk <= tq, 0.0, NEG)

    shared = dict(
        qkv_w_r=slabs(qkv_w),
        gate_w_r=slabs(gate_w),
        up_w_r=slabs(up_w),
        down_w_r=np.ascontiguousarray(np.asarray(down_w, f).reshape(16, P, D)),
        o_w_b=np.ascontiguousarray(
            np.asarray(o_w, f).reshape(8, P, D).astype(bf)),
        nthr_qkv=tg(qkv_thr, 24, True),
        gain_qkv=tg(qkv_gain, 24),
        nthr_gate=tg(gate_thr, 16, True),
        gain_gate=tg(gate_gain, 16),
        nthr_up=tg(up_thr, 16, True),
        gain_up=tg(up_gain, 16),
        norm1_wT=np.ascontiguousarray(np.asarray(norm1_w, f).reshape(8, P, 1)),
        norm2_wT=np.ascontiguousarray(np.asarray(norm2_w, f).reshape(8, P, 1)),
        maskc=maskc,
        ident=np.eye(P, dtype=f),
        ident_bf=np.eye(P, dtype=f).astype(bf),
        ones_bf=np.ones((P, 1), dtype=bf),
    )
    in_maps = []
    for c in range(N_CORES):
        m = dict(shared)
        m["x_shard"] = np.ascontiguousarray(
            np.concatenate([x[0, HB * c:HB * (c + 1)],
                            x[1, HB * c:HB * (c + 1)]], axis=0))
        in_maps.append(m)
    return in_maps


def kernel(**inputs):
    global LAST_EXEC_NS, LAST_RESULTS
    if "nc" not in _CACHE:
        _CACHE["nc"] = _build()
    nc = _CACHE["nc"]
    in_maps = _prep_inputs(**inputs)
    res = run_bass_kernel_spmd(nc, in_maps, core_ids=list(range(N_CORES)),
                               trace=TRACE)
    LAST_EXEC_NS = res.exec_time_ns
    LAST_RESULTS = res
    out = np.empty((B, T, D), np.float32)
    for c in range(N_CORES):
        sh = res.results[c]["out_shard"]
        out[0, HB * c:HB * (c + 1)] = sh[0:HB]
        out[1, HB * c:HB * (c + 1)] = sh[HB:TOK]
    return out


# revision 11
# speedup vs baseline: 1.6547x; 1.6547x over previous
"""DendriticFractalBlock on 8 trn2 NeuronCores (Bass/Tile, SPMD).

V2: token-parallel (256 tokens of each batch per core) for
norm/qkv/o_proj/FFN; head-parallel attention (4 heads x 1 batch per
core) bridged by 8-core AllToAlls (attention-out A2A split in two to
overlap o_proj). All matmul operands bf16 (f32 PSUM accumulation,
f32 softmax/norm/residual math). Weight/param DMAs are batched;
PSUM->SBUF copies ride the Vector engine, keeping ACT for
silu/exp only.
"""

import numpy as np

import concourse.bass as bass
import concourse.mybir as mybir
import concourse.tile as tile
import concourse.bacc as bacc
from concourse.bass_utils import run_bass_kernel_spmd

F32 = mybir.dt.float32
F32R = mybir.dt.float32r
BF16 = mybir.dt.bfloat16
AF = mybir.ActivationFunctionType
ALU = mybir.AluOpType

N_CORES = 8
RG = [list(range(8))]
B, T, D, FF = 2, 2048, 1024, 2048
ND = 4
TOK = 512          # tokens per core (256 per batch)
HB = 256           # per-batch tokens per core
P = 128
QKV = 3 * D
EPS = float(np.finfo(np.float32).eps)
NEG = -1.0e30

TRACE = False
LAST_EXEC_NS = None
LAST_RESULTS = None

_CACHE = {}


def _build():
    nc = bacc.Bacc("TRN2", target_bir_lowering=False, debug=False,
                   num_devices=N_CORES)

    x_d = nc.dram_tensor("x_shard", [TOK, D], F32, kind="ExternalInput")
    qkvw_d = nc.dram_tensor("qkv_w_r", [8, P, QKV], BF16, kind="ExternalInput")
    gatew_d = nc.dram_tensor("gate_w_r", [8, P, FF], BF16, kind="ExternalInput")
    upw_d = nc.dram_tensor("up_w_r", [8, P, FF], BF16, kind="ExternalInput")
    downw_d = nc.dram_tensor("down_w_r", [16, P, D], BF16, kind="ExternalInput")
    ow_d = nc.dram_tensor("o_w_b", [8, P, D], BF16, kind="ExternalInput")
    nthr_qkv_d = nc.dram_tensor("nthr_qkv", [P, 96], F32, kind="ExternalInput")
    gain_qkv_d = nc.dram_tensor("gain_qkv", [P, 96], F32, kind="ExternalInput")
    nthr_gate_d = nc.dram_tensor("nthr_gate", [P, 64], F32, kind="ExternalInput")
    gain_gate_d = nc.dram_tensor("gain_gate", [P, 64], F32, kind="ExternalInput")
    nthr_up_d = nc.dram_tensor("nthr_up", [P, 64], F32, kind="ExternalInput")
    gain_up_d = nc.dram_tensor("gain_up", [P, 64], F32, kind="ExternalInput")
    n1w_d = nc.dram_tensor("norm1_wT", [8, P, 1], F32, kind="ExternalInput")
    n2w_d = nc.dram_tensor("norm2_wT", [8, P, 1], F32, kind="ExternalInput")
    mask_d = nc.dram_tensor("maskc", [4, P, TOK], F32, kind="ExternalInput")
    ident_d = nc.dram_tensor("ident", [P, P], F32, kind="ExternalInput")
    identb_d = nc.dram_tensor("ident_bf", [P, P], BF16, kind="ExternalInput")
    onesb_d = nc.dram_tensor("ones_bf", [P, 1], BF16, kind="ExternalInput")
    out_d = nc.dram_tensor("out_shard", [TOK, D], F32, kind="ExternalOutput")

    with tile.TileContext(nc) as tc:
        with (
            tc.tile_pool(name="persist", bufs=1) as pp,
            tc.tile_pool(name="hT", bufs=1) as hp,
            tc.tile_pool(name="wq", bufs=20) as wqp,
            tc.tile_pool(name="scratch", bufs=2) as scp,
            tc.tile_pool(name="work", bufs=3) as wk,
            tc.tile_pool(name="small", bufs=4) as sm,
            tc.tile_pool(name="attn", bufs=17) as ap,
            tc.tile_pool(name="qkvh", bufs=2) as qp,
            tc.tile_pool(name="vn", bufs=20) as vp,
            tc.tile_pool(name="psum", bufs=8, space="PSUM") as ps,
            tc.tile_pool(name="dram", bufs=1, space="DRAM") as dp,
        ):
            # ---- constants ----
            ident = pp.tile([P, P], F32, tag="ident")
            nc.sync.dma_start(ident[:], ident_d[:])
            identb = pp.tile([P, P], BF16, tag="identb")
            nc.sync.dma_start(identb[:], identb_d[:])
            onesb = pp.tile([P, 1], BF16, tag="onesb")
            nc.sync.dma_start(onesb[:], onesb_d[:])
            epst = pp.tile([P, 1], F32, tag="epst")
            nc.vector.memset(epst[:], EPS)
            masks = []
            for i in range(4):
                m = pp.tile([P, TOK], F32, tag=f"mask{i}", name=f"mask{i}")
                nc.sync.dma_start(m[:], mask_d[i])
                masks.append(m)
            ntq = pp.tile([P, 96], F32, tag="ntq")
            nc.sync.dma_start(ntq[:], nthr_qkv_d[:])
            gnq = pp.tile([P, 96], F32, tag="gnq")
            nc.sync.dma_start(gnq[:], gain_qkv_d[:])
            ntg = pp.tile([P, 64], F32, tag="ntg")
            nc.sync.dma_start(ntg[:], nthr_gate_d[:])
            gng = pp.tile([P, 64], F32, tag="gng")
            nc.sync.dma_start(gng[:], gain_gate_d[:])
            ntu = pp.tile([P, 64], F32, tag="ntu")
            nc.sync.dma_start(ntu[:], nthr_up_d[:])
            gnu = pp.tile([P, 64], F32, tag="gnu")
            nc.sync.dma_start(gnu[:], gain_up_d[:])

            a2a1_in = dp.tile([8, 768, HB], BF16)
            a2a1_out = dp.tile([8, 768, HB], BF16)
            a2a2_in = [dp.tile([8, 128, HB], BF16, name=f"a2a2i{a}")
                       for a in range(2)]
            a2a2_out = [dp.tile([8, 128, HB], BF16, name=f"a2a2o{a}")
                        for a in range(2)]

            # ---- rmsnorm + transpose to feature-major (bf16) ----
            def norm_transpose(src_tiles, w_dram, tag):
                nw = pp.tile([P, 8], F32, tag=f"nw_{tag}", name=f"nw_{tag}")
                nc.sync.dma_start(nw[:], w_dram[:].rearrange("a p b -> p (a b)"))
                hT = [hp.tile([P, TOK], BF16, tag=f"hT{dc}", name=f"hT_{tag}{dc}")
                      for dc in range(8)]
                for mt in range(4):
                    xt = src_tiles[mt]
                    xn = scp.tile([P, D], F32, tag="xn")
                    ssq = sm.tile([P, 1], F32, tag="ssq")
                    nc.scalar.activation(xn[:], xt[:], AF.Square,
                                         accum_out=ssq[:])
                    rms = sm.tile([P, 1], F32, tag="rms")
                    nc.scalar.activation(rms[:], ssq[:], AF.Sqrt,
                                         scale=1.0 / D, bias=epst[:, 0:1])
                    rinv = sm.tile([P, 1], F32, tag="rinv")
                    nc.vector.reciprocal(rinv[:], rms[:])
                    nc.vector.tensor_scalar_mul(xn[:], xt[:], rinv[:, 0:1])
                    for dc in range(8):
                        tp = ps.tile([P, TOK], F32, tag="ps")
                        nc.tensor.transpose(tp[:, 0:P],
                                            xn[:, dc * P:(dc + 1) * P], ident[:])
                        nc.vector.tensor_scalar_mul(
                            hT[dc][:, mt * P:(mt + 1) * P], tp[:, 0:P],
                            nw[:, dc:dc + 1])
                return hT

            # ---- one dendritic col-tile (bf16 chain) ----
            def dend_tile(hT, wtiles, wcol, nthr, gain, pidx, mt):
                acc = wk.tile([P, TOK], BF16, tag="dacc", name=f"dacc{pidx}_{mt}")
                for n in range(ND):
                    pt = ps.tile([P, TOK], F32, tag="ps")
                    for kk in range(2):
                        nc.tensor.matmul(
                            pt[:], wtiles[2 * n + kk][:, wcol:wcol + P],
                            hT[2 * n + kk][:], start=(kk == 0), stop=(kk == 1))
                    st = wk.tile([P, TOK], BF16, tag="silu")
                    nc.scalar.activation(st[:], pt[:], AF.Silu,
                                         bias=nthr[:, pidx * ND + n:pidx * ND + n + 1])
                    if n == 0:
                        nc.vector.tensor_scalar_mul(
                            acc[:], st[:], gain[:, pidx * ND + n:pidx * ND + n + 1])
                    else:
                        g2 = wk.tile([P, TOK], BF16, tag="gained")
                        nc.vector.tensor_scalar_mul(
                            g2[:], st[:], gain[:, pidx * ND + n:pidx * ND + n + 1])
                        nc.vector.tensor_tensor(acc[:], acc[:], g2[:],
                                                op=ALU.max)
                return acc

            # ---- phase 1: x load + norm1 ----
            x_t = []
            with nc.named_scope("p1_norm1"):
                for mt in range(4):
                    xt = pp.tile([P, D], F32, tag=f"x{mt}", name=f"x{mt}")
                    nc.sync.dma_start(xt[:], x_d[mt * P:(mt + 1) * P, :])
                    x_t.append(xt)
                h1T = norm_transpose(x_t, n1w_d, "n1")

            # ---- phase 2: qkv dendritic -> a2a1_in ----
            scope_p2 = nc.named_scope("p2_qkv")
            scope_p2.__enter__()
            # stream qkv weights as (P, 1024) thirds: chunk (s, q3) covers
            # col-tiles 8*q3 .. 8*q3+7
            wcur = [None] * 8
            for mt in range(24):
                q3 = mt // 8
                if mt % 8 == 0:
                    wcur = []
                    for sidx in range(8):
                        wt = wqp.tile([P, 1024], BF16, tag="wq",
                                      name=f"wqk{q3}_{sidx}")
                        nc.sync.dma_start(
                            wt[:], qkvw_d[sidx, :, q3 * 1024:(q3 + 1) * 1024])
                        wcur.append(wt)
                acc = dend_tile(h1T, wcur, (mt % 8) * P, ntq, gnq, mt, 0)
                part, loc = mt // 8, mt % 8    # part: 0=q 1=k 2=v
                g = loc // 2
                r0 = 256 * part + (loc % 2) * P
                nc.sync.dma_start(a2a1_in[g, r0:r0 + P, :], acc[:, 0:HB])
                nc.sync.dma_start(a2a1_in[4 + g, r0:r0 + P, :], acc[:, HB:TOK])
            scope_p2.__exit__(None, None, None)

            # ---- phase 3: A2A #1 (q/k/v to head owners) ----
            with nc.named_scope("p3_a2a1"):
                nc.gpsimd.collective_compute(
                    "AllToAll", ALU.bypass, replica_groups=RG,
                    ins=[a2a1_in[:].opt()], outs=[a2a1_out[:].opt()],
                )

            # ---- phase 4: attention (4 local heads of one batch) ----
            scope_p4 = nc.named_scope("p4_attn")
            scope_p4.__enter__()
            for h in range(4):
                r = 64 * h
                # batched read: (64, j, {q,k,v}, 256) per chunk j
                qkvh = qp.tile([64, 8, 3, HB], BF16, tag="qkvh",
                               name=f"qkvh{h}")
                for j in range(8):
                    src = a2a1_out[j].rearrange("(s r) t -> r s t", s=3)
                    nc.sync.dma_start(qkvh[:, j, :, :], src[r:r + 64, :, :])
                vn = []
                for tkt in range(16):
                    j, inner = tkt // 2, tkt % 2
                    tp = ps.tile([P, 64], BF16, tag="ps")
                    nc.tensor.transpose(
                        tp[:], qkvh[:, j, 2, inner * P:(inner + 1) * P],
                        identb[0:64, 0:64])
                    vt = vp.tile([P, 65], BF16, tag="vn", name=f"vn{tkt}")
                    nc.vector.tensor_copy(vt[:, 0:64], tp[:])
                    nc.vector.tensor_copy(vt[:, 64:65], onesb[:])
                    vn.append(vt)
                for C in range(4):
                    n_tk = 4 * C + 4
                    at = []
                    for tkt in range(n_tk):
                        j, inner = tkt // 2, tkt % 2
                        sp = ps.tile([P, TOK], F32, tag="ps")
                        nc.tensor.matmul(
                            sp[:], qkvh[:, j, 1, inner * P:(inner + 1) * P],
                            qkvh[:, 2 * C:2 * C + 2, 0, :], start=True,
                            stop=True)
                        if tkt >= 4 * C:
                            nc.vector.tensor_tensor(sp[:], sp[:],
                                                    masks[tkt - 4 * C][:],
                                                    op=ALU.add)
                        a = ap.tile([P, TOK], BF16, tag="attnT",
                                    name=f"at{C}_{tkt}")
                        nc.scalar.activation(a[:], sp[:], AF.Exp, scale=0.125)
                        at.append(a)
                    po = ps.tile([65, TOK], F32, tag="ps", name=f"po{C}")
                    for tkt in range(n_tk):
                        nc.tensor.matmul(po[:], vn[tkt][:], at[tkt][:],
                                         start=(tkt == 0),
                                         stop=(tkt == n_tk - 1))
                    rden = sm.tile([1, TOK], F32, tag="rden")
                    nc.vector.reciprocal(rden[:], po[64:65, :])
                    rb = wk.tile([64, TOK], F32, tag="rb")
                    nc.gpsimd.partition_broadcast(rb[:], rden[:])
                    on = wk.tile([64, TOK], BF16, tag="onorm")
                    nc.vector.tensor_tensor(on[:], po[0:64, :], rb[:],
                                            op=ALU.mult)
                    aci = h // 2
                    rr = 64 * (h % 2)
                    nc.sync.dma_start(a2a2_in[aci][2 * C, rr:rr + 64, :],
                                      on[:, 0:HB])
                    nc.sync.dma_start(a2a2_in[aci][2 * C + 1, rr:rr + 64, :],
                                      on[:, HB:TOK])
                if h == 1:
                    with nc.named_scope("p5_a2a2a"):
                        nc.gpsimd.collective_compute(
                            "AllToAll", ALU.bypass, replica_groups=RG,
                            ins=[a2a2_in[0][:].opt()],
                            outs=[a2a2_out[0][:].opt()],
                        )
            scope_p4.__exit__(None, None, None)

            # ---- phase 5b: second half of attention-out exchange ----
            with nc.named_scope("p5_a2a2b"):
                nc.gpsimd.collective_compute(
                    "AllToAll", ALU.bypass, replica_groups=RG,
                    ins=[a2a2_in[1][:].opt()], outs=[a2a2_out[1][:].opt()],
                )

            # ---- phase 6: o_proj + residual -> xres; norm2 -> h2T ----
            scope_p6 = nc.named_scope("p6_oproj")
            scope_p6.__enter__()
            py = [[None, None] for _ in range(4)]
            for mt in range(4):
                for dn in range(2):
                    py[mt][dn] = ps.tile([P, TOK], F32, tag="ps",
                                         name=f"py{mt}{dn}")
            s_order = [0, 2, 4, 6, 1, 3, 5, 7]
            for si, s in enumerate(s_order):
                otl = wqp.tile([P, TOK], BF16, tag="wq", name=f"otl{s}")
                aci, j = s % 2, s // 2
                nc.sync.dma_start(otl[:, 0:HB], a2a2_out[aci][j])
                nc.sync.dma_start(otl[:, HB:TOK], a2a2_out[aci][4 + j])
                owt = wqp.tile([P, D], BF16, tag="wq", name=f"ow{s}")
                nc.sync.dma_start(owt[:], ow_d[s])
                for mt in range(4):
                    for dn in range(2):
                        nc.tensor.matmul(py[mt][dn][:],
                                         otl[:, mt * P:(mt + 1) * P],
                                         owt[:, dn * TOK:(dn + 1) * TOK],
                                         start=(si == 0), stop=(si == 7))
            xres = []
            for mt in range(4):
                xr = pp.tile([P, D], F32, tag=f"xr{mt}", name=f"xr{mt}")
                for dn in range(2):
                    cols = slice(dn * TOK, (dn + 1) * TOK)
                    nc.vector.tensor_tensor(xr[:, cols], py[mt][dn][:],
                                            x_t[mt][:, cols], op=ALU.add)
                xres.append(xr)
            h2T = norm_transpose(xres, n2w_d, "n2")
            scope_p6.__exit__(None, None, None)

            # ---- phase 7: gate/up dendritic (interleaved) -> ffT ----
            scope_p7 = nc.named_scope("p7_ffn")
            scope_p7.__enter__()
            ffT = [hp.tile([P, TOK], BF16, tag=f"ffT{mt}", name=f"ffT{mt}")
                   for mt in range(16)]
            gcur = [None] * 8
            ucur = [None] * 8
            for mt in range(16):
                if mt % 8 == 0:
                    h2i = mt // 8
                    gcur, ucur = [], []
                    for sidx in range(8):
                        gt = wqp.tile([P, 1024], BF16, tag="wq",
                                      name=f"wg{h2i}_{sidx}")
                        nc.sync.dma_start(
                            gt[:], gatew_d[sidx, :, h2i * 1024:(h2i + 1) * 1024])
                        gcur.append(gt)
                    for sidx in range(8):
                        ut = wqp.tile([P, 1024], BF16, tag="wq",
                                      name=f"wu{h2i}_{sidx}")
                        nc.sync.dma_start(
                            ut[:], upw_d[sidx, :, h2i * 1024:(h2i + 1) * 1024])
                        ucur.append(ut)
                acc_g = dend_tile(h2T, gcur, (mt % 8) * P, ntg, gng, mt, 1)
                acc_u = dend_tile(h2T, ucur, (mt % 8) * P, ntu, gnu, mt, 2)
                sg = wk.tile([P, TOK], BF16, tag="silu2")
                nc.scalar.activation(sg[:], acc_g[:], AF.Silu)
                nc.vector.tensor_tensor(ffT[mt][:], sg[:], acc_u[:],
                                        op=ALU.mult)
            scope_p7.__exit__(None, None, None)

            # ---- phase 8: down proj + residual -> out ----
            scope_p8 = nc.named_scope("p8_down")
            scope_p8.__enter__()
            pdt = [[None, None] for _ in range(4)]
            for mt in range(4):
                for dn in range(2):
                    pdt[mt][dn] = ps.tile([P, TOK], F32, tag="ps",
                                          name=f"pd{mt}{dn}")
            for s in range(16):
                dwt = wqp.tile([P, D], BF16, tag="wq", name=f"dw{s}")
                nc.sync.dma_start(dwt[:], downw_d[s])
                for mt in range(4):
                    for dn in range(2):
                        nc.tensor.matmul(pdt[mt][dn][:],
                                         ffT[s][:, mt * P:(mt + 1) * P],
                                         dwt[:, dn * TOK:(dn + 1) * TOK],
                                         start=(s == 0), stop=(s == 15))
            for mt in range(4):
                for dn in range(2):
                    cols = slice(dn * TOK, (dn + 1) * TOK)
                    of = wk.tile([P, TOK], F32, tag="outf")
                    nc.vector.tensor_tensor(of[:], pdt[mt][dn][:],
                                            xres[mt][:, cols], op=ALU.add)
                    nc.sync.dma_start(out_d[mt * P:(mt + 1) * P, cols], of[:])
            scope_p8.__exit__(None, None, None)

    nc.compile()
    return nc


def _prep_inputs(x, qkv_w, qkv_thr, qkv_gain, o_w, gate_w, gate_thr, gate_gain,
                 up_w, up_thr, up_gain, down_w, norm1_w, norm2_w):
    import ml_dtypes
    f = np.float32
    bf = ml_dtypes.bfloat16
    x = np.asarray(x, f)

    def slabs(w):
        w = np.asarray(w, f)
        n, dd, wid = w.shape
        return np.ascontiguousarray(w.reshape(n * 2, P, wid).astype(bf))

    def tg(v, n_mt, negate=False):
        # (ND, W) -> (P, n_mt*ND): [p, 4*mt+n] = v[n, 128*mt+p]
        v = np.asarray(v, f)
        if negate:
            v = -v
        r = np.transpose(v.reshape(ND, n_mt, P), (2, 1, 0))  # (P, n_mt, ND)
        return np.ascontiguousarray(r.reshape(P, n_mt * ND))

    maskc = np.zeros((4, P, TOK), f)
    for p in range(4):
        tk = P * p + np.arange(P)[:, None]
        tq = np.arange(TOK)[None, :]
        maskc[p] = np.where(tk <= tq, 0.0, NEG)

    shared = dict(
        qkv_w_r=slabs(qkv_w),
        gate_w_r=slabs(gate_w),
        up_w_r=slabs(up_w),
        down_w_r=np.ascontiguousarray(
            np.asarray(down_w, f).reshape(16, P, D).astype(bf)),
        o_w_b=np.ascontiguousarray(
            np.asarray(o_w, f).reshape(8, P, D).astype(bf)),
        nthr_qkv=tg(qkv_thr, 24, True),
        gain_qkv=tg(qkv_gain, 24),
        nthr_gate=tg(gate_thr, 16, True),
        gain_gate=tg(gate_gain, 16),
        nthr_up=tg(up_thr, 16, True),
        gain_up=tg(up_gain, 16),
        norm1_wT=np.ascontiguousarray(np.asarray(norm1_w, f).reshape(8, P, 1)),
        norm2_wT=np.ascontiguousarray(np.asarray(norm2_w, f).reshape(8, P, 1)),
        maskc=maskc,
        ident=np.eye(P, dtype=f),
        ident_bf=np.eye(P, dtype=f).astype(bf),
        ones_bf=np.ones((P, 1), dtype=bf),
    )
    in_maps = []
    for c in range(N_CORES):
        m = dict(shared)
        m["x_shard"] = np.ascontiguousarray(
            np.concatenate([x[0, HB * c:HB * (c + 1)],
                            x[1, HB * c:HB * (c + 1)]], axis=0))
        in_maps.append(m)
    return in_maps


def kernel(**inputs):
    global LAST_EXEC_NS, LAST_RESULTS
    if "nc" not in _CACHE:
        _CACHE["nc"] = _build()
    nc = _CACHE["nc"]
    in_maps = _prep_inputs(**inputs)
    res = run_bass_kernel_spmd(nc, in_maps, core_ids=list(range(N_CORES)),
                               trace=TRACE)
    LAST_EXEC_NS = res.exec_time_ns
    LAST_RESULTS = res
    out = np.empty((B, T, D), np.float32)
    for c in range(N_CORES):
        sh = res.results[c]["out_shard"]
        out[0, HB * c:HB * (c + 1)] = sh[0:HB]
        out[1, HB * c:HB * (c + 1)] = sh[HB:TOK]
    return out


# revision 17
# speedup vs baseline: 1.6903x; 1.0215x over previous
"""DendriticFractalBlock on 8 trn2 NeuronCores (Bass/Tile, SPMD).

V2: token-parallel (256 tokens of each batch per core) for
norm/qkv/o_proj/FFN; head-parallel attention (4 heads x 1 batch per
core) bridged by 8-core AllToAlls (attention-out A2A split in two to
overlap o_proj). All matmul operands bf16 (f32 PSUM accumulation,
f32 softmax/norm/residual math). Weight/param DMAs are batched;
PSUM->SBUF copies ride the Vector engine, keeping ACT for
silu/exp only.
"""

import numpy as np

import concourse.bass as bass
import concourse.mybir as mybir
import concourse.tile as tile
import concourse.bacc as bacc
from concourse.bass_utils import run_bass_kernel_spmd

F32 = mybir.dt.float32
F32R = mybir.dt.float32r
BF16 = mybir.dt.bfloat16
AF = mybir.ActivationFunctionType
ALU = mybir.AluOpType

N_CORES = 8
RG = [list(range(8))]
B, T, D, FF = 2, 2048, 1024, 2048
ND = 4
TOK = 512          # tokens per core (256 per batch)
HB = 256           # per-batch tokens per core
P = 128
QKV = 3 * D
EPS = float(np.finfo(np.float32).eps)
NEG = -1.0e30

TRACE = False
LAST_EXEC_NS = None
LAST_RESULTS = None

_CACHE = {}


def _build():
    nc = bacc.Bacc("TRN2", target_bir_lowering=False, debug=False,
                   num_devices=N_CORES)

    x_d = nc.dram_tensor("x_shard", [TOK, D], F32, kind="ExternalInput")
    qkvw_d = nc.dram_tensor("qkv_w_r", [8, P, QKV], BF16, kind="ExternalInput")
    gatew_d = nc.dram_tensor("gate_w_r", [8, P, FF], BF16, kind="ExternalInput")
    upw_d = nc.dram_tensor("up_w_r", [8, P, FF], BF16, kind="ExternalInput")
    downw_d = nc.dram_tensor("down_w_r", [16, P, D], BF16, kind="ExternalInput")
    ow_d = nc.dram_tensor("o_w_b", [8, P, D], BF16, kind="ExternalInput")
    nthr_qkv_d = nc.dram_tensor("nthr_qkv", [P, 96], F32, kind="ExternalInput")
    gain_qkv_d = nc.dram_tensor("gain_qkv", [P, 96], F32, kind="ExternalInput")
    nthr_gate_d = nc.dram_tensor("nthr_gate", [P, 64], F32, kind="ExternalInput")
    gain_gate_d = nc.dram_tensor("gain_gate", [P, 64], F32, kind="ExternalInput")
    nthr_up_d = nc.dram_tensor("nthr_up", [P, 64], F32, kind="ExternalInput")
    gain_up_d = nc.dram_tensor("gain_up", [P, 64], F32, kind="ExternalInput")
    n1w_d = nc.dram_tensor("norm1_wT", [8, P, 1], F32, kind="ExternalInput")
    n2w_d = nc.dram_tensor("norm2_wT", [8, P, 1], F32, kind="ExternalInput")
    mask_d = nc.dram_tensor("maskc", [4, P, TOK], F32, kind="ExternalInput")
    ident_d = nc.dram_tensor("ident", [P, P], F32, kind="ExternalInput")
    identb_d = nc.dram_tensor("ident_bf", [P, P], BF16, kind="ExternalInput")
    onesb_d = nc.dram_tensor("ones_bf", [P, 1], BF16, kind="ExternalInput")
    out_d = nc.dram_tensor("out_shard", [TOK, D], F32, kind="ExternalOutput")

    with tile.TileContext(nc) as tc:
        with (
            tc.tile_pool(name="persist", bufs=1) as pp,
            tc.tile_pool(name="hT", bufs=1) as hp,
            tc.tile_pool(name="wq", bufs=18) as wqp,
            tc.tile_pool(name="scratch", bufs=1) as scp,
            tc.tile_pool(name="work", bufs=3) as wk,
            tc.tile_pool(name="small", bufs=4) as sm,
            tc.tile_pool(name="attn", bufs=28) as ap,
            tc.tile_pool(name="qkvh", bufs=2) as qp,
            tc.tile_pool(name="vn", bufs=32) as vp,
            tc.tile_pool(name="psum", bufs=8, space="PSUM") as ps,
            tc.tile_pool(name="dram", bufs=1, space="DRAM") as dp,
        ):
            # ---- constants ----
            ident = pp.tile([P, P], F32, tag="ident")
            nc.sync.dma_start(ident[:], ident_d[:])
            identb = pp.tile([P, P], BF16, tag="identb")
            nc.sync.dma_start(identb[:], identb_d[:])
            onesb = pp.tile([P, 1], BF16, tag="onesb")
            nc.sync.dma_start(onesb[:], onesb_d[:])
            epst = pp.tile([P, 1], F32, tag="epst")
            nc.vector.memset(epst[:], EPS)
            masks = []
            for i in range(4):
                m = pp.tile([P, TOK], F32, tag=f"mask{i}", name=f"mask{i}")
                nc.sync.dma_start(m[:], mask_d[i])
                masks.append(m)
            ntq = pp.tile([P, 96], F32, tag="ntq")
            nc.sync.dma_start(ntq[:], nthr_qkv_d[:])
            gnq = pp.tile([P, 96], F32, tag="gnq")
            nc.sync.dma_start(gnq[:], gain_qkv_d[:])
            ntg = pp.tile([P, 64], F32, tag="ntg")
            nc.sync.dma_start(ntg[:], nthr_gate_d[:])
            gng = pp.tile([P, 64], F32, tag="gng")
            nc.sync.dma_start(gng[:], gain_gate_d[:])
            ntu = pp.tile([P, 64], F32, tag="ntu")
            nc.sync.dma_start(ntu[:], nthr_up_d[:])
            gnu = pp.tile([P, 64], F32, tag="gnu")
            nc.sync.dma_start(gnu[:], gain_up_d[:])

            a2a1_in = [dp.tile([8, 384, HB], BF16, name=f"a2a1i{a}")
                       for a in range(2)]
            a2a1_out = [dp.tile([8, 384, HB], BF16, name=f"a2a1o{a}")
                        for a in range(2)]
            a2a2_in = [dp.tile([8, 128, HB], BF16, name=f"a2a2i{a}")
                       for a in range(2)]
            a2a2_out = [dp.tile([8, 128, HB], BF16, name=f"a2a2o{a}")
                        for a in range(2)]

            # ---- rmsnorm + transpose to feature-major (bf16) ----
            def norm_transpose(src_tiles, w_dram, tag):
                nw = pp.tile([P, 8], F32, tag=f"nw_{tag}", name=f"nw_{tag}")
                nc.sync.dma_start(nw[:], w_dram[:].rearrange("a p b -> p (a b)"))
                hT = [hp.tile([P, TOK], BF16, tag=f"hT{dc}", name=f"hT_{tag}{dc}")
                      for dc in range(8)]
                for mt in range(4):
                    xt = src_tiles[mt]
                    xn = scp.tile([P, D], F32, tag="xn")
                    ssq = sm.tile([P, 1], F32, tag="ssq")
                    nc.scalar.activation(xn[:], xt[:], AF.Square,
                                         accum_out=ssq[:])
                    rms = sm.tile([P, 1], F32, tag="rms")
                    nc.scalar.activation(rms[:], ssq[:], AF.Sqrt,
                                         scale=1.0 / D, bias=epst[:, 0:1])
                    rinv = sm.tile([P, 1], F32, tag="rinv")
                    nc.vector.reciprocal(rinv[:], rms[:])
                    nc.vector.tensor_scalar_mul(xn[:], xt[:], rinv[:, 0:1])
                    for dc in range(8):
                        tp = ps.tile([P, TOK], F32, tag="ps")
                        nc.tensor.transpose(tp[:, 0:P],
                                            xn[:, dc * P:(dc + 1) * P], ident[:])
                        nc.vector.tensor_scalar_mul(
                            hT[dc][:, mt * P:(mt + 1) * P], tp[:, 0:P],
                            nw[:, dc:dc + 1])
                return hT

            # ---- one dendritic col-tile (bf16 chain) ----
            def dend_tile(hT, wtiles, wcol, nthr, gain, pidx, mt):
                acc = wk.tile([P, TOK], BF16, tag="dacc", name=f"dacc{pidx}_{mt}")
                for n in range(ND):
                    pt = ps.tile([P, TOK], F32, tag="ps")
                    for kk in range(2):
                        nc.tensor.matmul(
                            pt[:], wtiles[2 * n + kk][:, wcol:wcol + P],
                            hT[2 * n + kk][:], start=(kk == 0), stop=(kk == 1))
                    st = wk.tile([P, TOK], BF16, tag="silu")
                    nc.scalar.activation(st[:], pt[:], AF.Silu,
                                         bias=nthr[:, pidx * ND + n:pidx * ND + n + 1])
                    if n == 0:
                        nc.vector.tensor_scalar_mul(
                            acc[:], st[:], gain[:, pidx * ND + n:pidx * ND + n + 1])
                    else:
                        g2 = wk.tile([P, TOK], BF16, tag="gained")
                        nc.vector.tensor_scalar_mul(
                            g2[:], st[:], gain[:, pidx * ND + n:pidx * ND + n + 1])
                        nc.vector.tensor_tensor(acc[:], acc[:], g2[:],
                                                op=ALU.max)
                return acc

            # ---- phase 1: x load + norm1 ----
            x_t = []
            with nc.named_scope("p1_norm1"):
                for mt in range(4):
                    xt = pp.tile([P, D], F32, tag=f"x{mt}", name=f"x{mt}")
                    nc.sync.dma_start(xt[:], x_d[mt * P:(mt + 1) * P, :])
                    x_t.append(xt)
                h1T = norm_transpose(x_t, n1w_d, "n1")

            # ---- phase 2: qkv dendritic -> a2a1_in ----
            scope_p2 = nc.named_scope("p2_qkv")
            scope_p2.__enter__()
            # two passes (even then odd col-tiles); each pass streams qkv
            # weights as (P, 512) chunks (s, quad) covering its 2 m-tiles
            for passi in range(2):
                mts = [m for m in range(24) if m % 2 == passi]
                wcur = {}
                for mt in mts:
                    q4 = mt // 4
                    if q4 not in wcur:
                        wcur = {q4: []}
                        for sidx in range(8):
                            wt = wqp.tile([P, TOK], BF16, tag="wq",
                                          name=f"wqk{passi}_{q4}_{sidx}")
                            nc.sync.dma_start(
                                wt[:], qkvw_d[sidx, :, q4 * TOK:(q4 + 1) * TOK])
                            wcur[q4].append(wt)
                    acc = dend_tile(h1T, wcur[q4], (mt % 4) * P, ntq, gnq,
                                    mt, 0)
                    part, loc = mt // 8, mt % 8    # part: 0=q 1=k 2=v
                    g = loc // 2
                    r0 = P * part
                    nc.sync.dma_start(a2a1_in[passi][g, r0:r0 + P, :],
                                      acc[:, 0:HB])
                    nc.sync.dma_start(a2a1_in[passi][4 + g, r0:r0 + P, :],
                                      acc[:, HB:TOK])
                with nc.named_scope(f"p3_a2a1{'ab'[passi]}"):
                    nc.gpsimd.collective_compute(
                        "AllToAll", ALU.bypass, replica_groups=RG,
                        ins=[a2a1_in[passi][:].opt()],
                        outs=[a2a1_out[passi][:].opt()],
                    )
            scope_p2.__exit__(None, None, None)

            # ---- phase 4: attention (pipelined across (head, chunk)) ----
            scope_p4 = nc.named_scope("p4_attn")
            scope_p4.__enter__()

            def head_setup(h):
                a, hl = h // 2, h % 2
                qkvh = qp.tile([64, 8, 3, HB], BF16, tag="qkvh",
                               name=f"qkvh{h}")
                for j in range(8):
                    src = a2a1_out[a][j].rearrange("(s r) t -> r s t", s=3)
                    nc.sync.dma_start(qkvh[:, j, :, :],
                                      src[64 * hl:64 * hl + 64, :, :])
                vn = []
                for tkt in range(16):
                    j, inner = tkt // 2, tkt % 2
                    tp = ps.tile([P, 64], BF16, tag="ps")
                    nc.tensor.transpose(
                        tp[:], qkvh[:, j, 2, inner * P:(inner + 1) * P],
                        identb[0:64, 0:64])
                    vt = vp.tile([P, 65], BF16, tag="vn", name=f"vn{h}_{tkt}")
                    nc.vector.tensor_copy(vt[:, 0:64], tp[:])
                    nc.vector.tensor_copy(vt[:, 64:65], onesb[:])
                    vn.append(vt)
                return qkvh, vn

            def scores_chain(h, C, qkvh):
                n_tk = 4 * C + 4
                at = []
                for tkt in range(n_tk):
                    j, inner = tkt // 2, tkt % 2
                    sp = ps.tile([P, TOK], F32, tag="ps")
                    nc.tensor.matmul(
                        sp[:], qkvh[:, j, 1, inner * P:(inner + 1) * P],
                        qkvh[:, 2 * C:2 * C + 2, 0, :], start=True, stop=True)
                    if tkt >= 4 * C:
                        nc.vector.tensor_tensor(sp[:], sp[:],
                                                masks[tkt - 4 * C][:],
                                                op=ALU.add)
                    a = ap.tile([P, TOK], BF16, tag="attnT",
                                name=f"at{h}_{C}_{tkt}")
                    nc.scalar.activation(a[:], sp[:], AF.Exp, scale=0.125)
                    at.append(a)
                return at

            def pv_norm(h, C, at, vn):
                n_tk = len(at)
                po = ps.tile([65, TOK], F32, tag="ps", name=f"po{h}_{C}")
                for tkt in range(n_tk):
                    nc.tensor.matmul(po[:], vn[tkt][:], at[tkt][:],
                                     start=(tkt == 0), stop=(tkt == n_tk - 1))
                rden = sm.tile([1, TOK], F32, tag="rden")
                nc.vector.reciprocal(rden[:], po[64:65, :])
                rb = wk.tile([64, TOK], F32, tag="rb")
                nc.gpsimd.partition_broadcast(rb[:], rden[:])
                on = wk.tile([64, TOK], BF16, tag="onorm")
                nc.vector.tensor_tensor(on[:], po[0:64, :], rb[:],
                                        op=ALU.mult)
                aci = h // 2
                rr = 64 * (h % 2)
                nc.sync.dma_start(a2a2_in[aci][2 * C, rr:rr + 64, :],
                                  on[:, 0:HB])
                nc.sync.dma_start(a2a2_in[aci][2 * C + 1, rr:rr + 64, :],
                                  on[:, HB:TOK])

            prev = None
            for h in range(4):
                cur_qkvh, cur_vn = head_setup(h)
                for C in range(4):
                    at = scores_chain(h, C, cur_qkvh)
                    if prev is not None:
                        pv_norm(*prev)
                        if prev[0] == 1 and prev[1] == 3:
                            with nc.named_scope("p5_a2a2a"):
                                nc.gpsimd.collective_compute(
                                    "AllToAll", ALU.bypass, replica_groups=RG,
                                    ins=[a2a2_in[0][:].opt()],
                                    outs=[a2a2_out[0][:].opt()],
                                )
                    prev = (h, C, at, cur_vn)
            pv_norm(*prev)
            scope_p4.__exit__(None, None, None)

            # ---- phase 5b: second half of attention-out exchange ----
            with nc.named_scope("p5_a2a2b"):
                nc.gpsimd.collective_compute(
                    "AllToAll", ALU.bypass, replica_groups=RG,
                    ins=[a2a2_in[1][:].opt()], outs=[a2a2_out[1][:].opt()],
                )

            # ---- phase 6: o_proj + residual -> xres; norm2 -> h2T ----
            scope_p6 = nc.named_scope("p6_oproj")
            scope_p6.__enter__()
            py = [[None, None] for _ in range(4)]
            for mt in range(4):
                for dn in range(2):
                    py[mt][dn] = ps.tile([P, TOK], F32, tag="ps",
                                         name=f"py{mt}{dn}")
            s_order = [0, 2, 4, 6, 1, 3, 5, 7]
            for si, s in enumerate(s_order):
                otl = wqp.tile([P, TOK], BF16, tag="wq", name=f"otl{s}")
                aci, j = s % 2, s // 2
                nc.sync.dma_start(otl[:, 0:HB], a2a2_out[aci][j])
                nc.sync.dma_start(otl[:, HB:TOK], a2a2_out[aci][4 + j])
                owt = wqp.tile([P, D], BF16, tag="wq", name=f"ow{s}")
                nc.sync.dma_start(owt[:], ow_d[s])
                for mt in range(4):
                    for dn in range(2):
                        nc.tensor.matmul(py[mt][dn][:],
                                         otl[:, mt * P:(mt + 1) * P],
                                         owt[:, dn * TOK:(dn + 1) * TOK],
                                         start=(si == 0), stop=(si == 7))
            xres = []
            for mt in range(4):
                xr = pp.tile([P, D], F32, tag=f"xr{mt}", name=f"xr{mt}")
                for dn in range(2):
                    cols = slice(dn * TOK, (dn + 1) * TOK)
                    nc.vector.tensor_tensor(xr[:, cols], py[mt][dn][:],
                                            x_t[mt][:, cols], op=ALU.add)
                xres.append(xr)
            h2T = norm_transpose(xres, n2w_d, "n2")
            scope_p6.__exit__(None, None, None)

            # ---- phase 7: gate/up dendritic (interleaved) -> ffT ----
            scope_p7 = nc.named_scope("p7_ffn")
            scope_p7.__enter__()
            ffT = [hp.tile([P, TOK], BF16, tag=f"ffT{mt}", name=f"ffT{mt}")
                   for mt in range(16)]
            gcur = [None] * 8
            ucur = [None] * 8
            for mt in range(16):
                if mt % 8 == 0:
                    h2i = mt // 8
                    gcur, ucur = [], []
                    for sidx in range(8):
                        gt = wqp.tile([P, 1024], BF16, tag="wq",
                                      name=f"wg{h2i}_{sidx}")
                        nc.sync.dma_start(
                            gt[:], gatew_d[sidx, :, h2i * 1024:(h2i + 1) * 1024])
                        gcur.append(gt)
                    for sidx in range(8):
                        ut = wqp.tile([P, 1024], BF16, tag="wq",
                                      name=f"wu{h2i}_{sidx}")
                        nc.sync.dma_start(
                            ut[:], upw_d[sidx, :, h2i * 1024:(h2i + 1) * 1024])
                        ucur.append(ut)
                acc_g = dend_tile(h2T, gcur, (mt % 8) * P, ntg, gng, mt, 1)
                acc_u = dend_tile(h2T, ucur, (mt % 8) * P, ntu, gnu, mt, 2)
                sg = wk.tile([P, TOK], BF16, tag="silu2")
                nc.scalar.activation(sg[:], acc_g[:], AF.Silu)
                nc.vector.tensor_tensor(ffT[mt][:], sg[:], acc_u[:],
                                        op=ALU.mult)
            scope_p7.__exit__(None, None, None)

            # ---- phase 8: down proj + residual -> out ----
            scope_p8 = nc.named_scope("p8_down")
            scope_p8.__enter__()
            pdt = [[None, None] for _ in range(4)]
            for mt in range(4):
                for dn in range(2):
                    pdt[mt][dn] = ps.tile([P, TOK], F32, tag="ps",
                                          name=f"pd{mt}{dn}")
            for s in range(16):
                dwt = wqp.tile([P, D], BF16, tag="wq", name=f"dw{s}")
                nc.sync.dma_start(dwt[:], downw_d[s])
                for mt in range(4):
                    for dn in range(2):
                        nc.tensor.matmul(pdt[mt][dn][:],
                                         ffT[s][:, mt * P:(mt + 1) * P],
                                         dwt[:, dn * TOK:(dn + 1) * TOK],
                                         start=(s == 0), stop=(s == 15))
            for mt in range(4):
                for dn in range(2):
                    cols = slice(dn * TOK, (dn + 1) * TOK)
                    of = wk.tile([P, TOK], F32, tag="outf")
                    nc.vector.tensor_tensor(of[:], pdt[mt][dn][:],
                                            xres[mt][:, cols], op=ALU.add)
                    nc.sync.dma_start(out_d[mt * P:(mt + 1) * P, cols], of[:])
            scope_p8.__exit__(None, None, None)

    nc.compile()
    return nc


def _prep_inputs(x, qkv_w, qkv_thr, qkv_gain, o_w, gate_w, gate_thr, gate_gain,
                 up_w, up_thr, up_gain, down_w, norm1_w, norm2_w):
    import ml_dtypes
    f = np.float32
    bf = ml_dtypes.bfloat16
    x = np.asarray(x, f)

    def slabs(w):
        w = np.asarray(w, f)
        n, dd, wid = w.shape
        return np.ascontiguousarray(w.reshape(n * 2, P, wid).astype(bf))

    def tg(v, n_mt, negate=False):
        # (ND, W) -> (P, n_mt*ND): [p, 4*mt+n] = v[n, 128*mt+p]
        v = np.asarray(v, f)
        if negate:
            v = -v
        r = np.transpose(v.reshape(ND, n_mt, P), (2, 1, 0))  # (P, n_mt, ND)
        return np.ascontiguousarray(r.reshape(P, n_mt * ND))

    maskc = np.zeros((4, P, TOK), f)
    for p in range(4):
        tk = P * p + np.arange(P)[:, None]
        tq = np.arange(TOK)[None, :]
        maskc[p] = np.where(tk <= tq, 0.0, NEG)

    shared = dict(
        qkv_w_r=slabs(qkv_w),
        gate_w_r=slabs(gate_w),
        up_w_r=slabs(up_w),
        down_w_r=np.ascontiguousarray(
            np.asarray(down_w, f).reshape(16, P, D).astype(bf)),
        o_w_b=np.ascontiguousarray(
            np.asarray(o_w, f).reshape(8, P, D).astype(bf)),
        nthr_qkv=tg(qkv_thr, 24, True),
        gain_qkv=tg(qkv_gain, 24),
        nthr_gate=tg(gate_thr, 16, True),
        gain_gate=tg(gate_gain, 16),
        nthr_up=tg(up_thr, 16, True),
        gain_up=tg(up_gain, 16),
        norm1_wT=np.ascontiguousarray(np.asarray(norm1_w, f).reshape(8, P, 1)),
        norm2_wT=np.ascontiguousarray(np.asarray(norm2_w, f).reshape(8, P, 1)),
        maskc=maskc,
        ident=np.eye(P, dtype=f),
        ident_bf=np.eye(P, dtype=f).astype(bf),
        ones_bf=np.ones((P, 1), dtype=bf),
    )
    in_maps = []
    for c in range(N_CORES):
        m = dict(shared)
        m["x_shard"] = np.ascontiguousarray(
            np.concatenate([x[0, HB * c:HB * (c + 1)],
                            x[1, HB * c:HB * (c + 1)]], axis=0))
        in_maps.append(m)
    return in_maps


def kernel(**inputs):
    global LAST_EXEC_NS, LAST_RESULTS
    if "nc" not in _CACHE:
        _CACHE["nc"] = _build()
    nc = _CACHE["nc"]
    in_maps = _prep_inputs(**inputs)
    res = run_bass_kernel_spmd(nc, in_maps, core_ids=list(range(N_CORES)),
                               trace=TRACE)
    LAST_EXEC_NS = res.exec_time_ns
    LAST_RESULTS = res
    out = np.empty((B, T, D), np.float32)
    for c in range(N_CORES):
        sh = res.results[c]["out_shard"]
        out[0, HB * c:HB * (c + 1)] = sh[0:HB]
        out[1, HB * c:HB * (c + 1)] = sh[HB:TOK]
    return out
